# revision 16
# baseline (speedup 1.0000x reference)
"""AdaptiveCoverageAttention TRN2 kernel: 8-way (batch x head-group) sharded.

Sharding: core c in 0..7 -> batch b = c//4, head-group hg = c%4 (4 heads each).
Each core computes its 4 heads' attention + its partial output projection;
the host sums the 4 partials per batch (and adds b_out). No collectives.

v3: the attention exp stream (ScalarE-bound, ~1.1us per [128,1024] exp tile)
runs nearly the whole kernel; everything else hides inside its PE slack.
- Prefix: dual-ring DMA (sync+scalar DGE), K^T for pair 0, Q^T for (p0,ic0),
  first two V tiles, and the gate/coverage MLPs (pooled sums reduced per
  DMA chunk so they pipeline with the loads).
- Stream: per j-tile, S^T (K=64 row slices), exp with per-partition coverage
  bias, AV with M=64 V tiles, and softmax denominators as concurrent M=1
  col-tiles into one PSUM bank (partitions 0/32/64/96). One "job" (V tile,
  K-p1/Q chunk, or ic0 out-proj half) is emitted per j-tile to fill slack.
- PSUM: S 2x[128,1024] (4 banks) + AV pair accumulator (2) + denominators
  (1) + job scratch (1) = 8 banks exactly.
- Tail: remaining out-projection with a wide PSUM pool, bf16 output DMA.
"""
import sys

sys.path.insert(0, "/opt/trn_rl_repo")

import numpy as np

B, N, D, H = 2, 2048, 1024, 16
HD = D // H            # 64
HPC = 4                # heads per core
NCORES = 8
IC_W = 1024

_COMPILED = {}


def _bf16(x):
    import ml_dtypes
    return np.ascontiguousarray(np.asarray(x, np.float32)).astype(ml_dtypes.bfloat16)


def build(n=N):
    """Build the per-core Bass graph for sequence length n (n % 1024 == 0)."""
    import contextlib

    import concourse.bacc as bacc
    import concourse.tile as tile
    from concourse import mybir

    f32 = mybir.dt.float32
    bf16 = mybir.dt.bfloat16
    AFT = mybir.ActivationFunctionType

    NJ = n // 128          # 16 j-tiles (key positions)
    NI = n // 512          # 4 512-wide chunks (projection phase)
    NI2 = n // IC_W        # 2 i-chunks (query positions, attention phase)
    DC = D // 128          # 8 contraction chunks

    nc = bacc.Bacc("TRN2", target_bir_lowering=False, debug=False,
                   num_devices=NCORES)

    dram = lambda name, shape, dt, kind: nc.dram_tensor(name, shape, dt, kind=kind).ap()
    XT = dram("xT", [D, n], bf16, "ExternalInput")
    WQK = dram("wqk", [D, 512], bf16, "ExternalInput")
    WV = dram("wv", [D, 256], bf16, "ExternalInput")
    WO = dram("wo", [256, D], bf16, "ExternalInput")
    COVT = dram("covT", [1, n], bf16, "ExternalInput")
    WCE1 = dram("wce1", [1, 256], bf16, "ExternalInput")
    BCE1 = dram("bce1", [128, 2], f32, "ExternalInput")
    WCE2 = dram("wce2", [128, 8], bf16, "ExternalInput")
    BCE2 = dram("bce2", [128, 4], f32, "ExternalInput")
    WFG1 = dram("wfg1", [D, 256], f32, "ExternalInput")
    BFG1 = dram("bfg1", [128, 2], f32, "ExternalInput")
    WFG2 = dram("wfg2", [128, 2], f32, "ExternalInput")
    BFG2 = dram("bfg2", [1, 1], f32, "ExternalInput")
    OUT = dram("out", [n, D], bf16, "ExternalOutput")

    with tile.TileContext(nc) as tc, contextlib.ExitStack() as ctx:
        consts = ctx.enter_context(tc.tile_pool(name="consts", bufs=1))
        xtp = ctx.enter_context(tc.tile_pool(name="xtp", bufs=DC))
        qkv = ctx.enter_context(tc.tile_pool(name="qkv", bufs=1))
        big2 = ctx.enter_context(tc.tile_pool(name="big2", bufs=1))
        ep = ctx.enter_context(tc.tile_pool(name="ep", bufs=6))
        rp = ctx.enter_context(tc.tile_pool(name="rp", bufs=3))
        wfp = ctx.enter_context(tc.tile_pool(name="wfp", bufs=8))
        yp = ctx.enter_context(tc.tile_pool(name="yp", bufs=4))

        # ---- constants into SBUF (split across both DGE rings) ----
        wqk_sb = consts.tile([128, DC, 512], bf16)
        wv_sb = consts.tile([128, DC, 256], bf16)
        wo_sb = consts.tile([128, 2, D], bf16)
        covT_sb = consts.tile([1, n], bf16)
        wce1_sb = consts.tile([1, 256], bf16)
        bce1_sb = consts.tile([128, 2], f32)
        wce2_sb = consts.tile([128, 8], bf16)
        bce2_sb = consts.tile([128, 4], f32)
        bfg1_sb = consts.tile([128, 2], f32)
        wfg2_sb = consts.tile([128, 2], f32)
        bfg2_sb = consts.tile([1, 1], f32)
        for dc in range(DC):
            nc.sync.dma_start(out=wqk_sb[:, dc, :], in_=WQK[dc * 128:(dc + 1) * 128, :])
            nc.scalar.dma_start(out=wv_sb[:, dc, :], in_=WV[dc * 128:(dc + 1) * 128, :])
        for pt in range(2):
            nc.scalar.dma_start(out=wo_sb[:, pt, :], in_=WO[pt * 128:(pt + 1) * 128, :])
        nc.scalar.dma_start(out=covT_sb, in_=COVT)
        nc.scalar.dma_start(out=wce1_sb, in_=WCE1)
        nc.scalar.dma_start(out=bce1_sb, in_=BCE1)
        nc.scalar.dma_start(out=wce2_sb, in_=WCE2)
        nc.scalar.dma_start(out=bce2_sb, in_=BCE2)
        nc.scalar.dma_start(out=bfg1_sb, in_=BFG1)
        nc.scalar.dma_start(out=wfg2_sb, in_=WFG2)
        nc.scalar.dma_start(out=bfg2_sb, in_=BFG2)

        ones_f = consts.tile([1, 128], f32)
        nc.vector.memset(ones_f, 1.0)
        ones_bf = consts.tile([128, 1], bf16)
        nc.vector.memset(ones_bf, 1.0)

        pooled4 = consts.tile([128, DC, NI], f32)
        pooled_sb = consts.tile([128, DC], f32)
        hidg_sb = consts.tile([128, 2], f32)
        g_sb = consts.tile([1, 1], f32)
        gb_sb = consts.tile([128, 1], f32)
        bias_sb = consts.tile([128, NJ, 4], f32)

        # ---- xT DMA: j-chunk-major, alternating DGE rings; pooled partial
        #      sums reduced per chunk so they pipeline with the loads ----
        xts = []
        for dc in range(DC):
            xt = xtp.tile([128, NI, 512], bf16, tag="xt", name=f"xt{dc}")
            xts.append(xt)
        for jc in range(NI):
            for dc in range(DC):
                eng = nc.sync if dc % 2 == 0 else nc.scalar
                eng.dma_start(out=xts[dc][:, jc, :],
                              in_=XT[dc * 128:(dc + 1) * 128,
                                     jc * 512:(jc + 1) * 512])
            for dc in range(DC):
                nc.vector.reduce_sum(pooled4[:, dc, jc:jc + 1], xts[dc][:, jc, :],
                                     axis=mybir.AxisListType.X)
        for dc in range(DC):
            nc.vector.reduce_sum(pooled_sb[:, dc:dc + 1], pooled4[:, dc, :],
                                 axis=mybir.AxisListType.X)

        qt_sb = qkv.tile([128, 2, n], bf16)
        ktp_sb = qkv.tile([128, 2, n], bf16)
        vsb = qkv.tile([128, NJ, 4, 64], bf16)

        # ---- prefix: K^T(p0), Q^T(p0, ic0), V(0..1), MLPs ----
        with tc.tile_pool(name="pfA", bufs=3, space="PSUM") as pfA, \
             tc.tile_pool(name="pft", bufs=1, space="PSUM") as pft:

            def qk_chunk(pool, cb, ic, tag="qk", bufs=2):
                pq = pool.tile([128, 512], f32, tag=tag, name=f"pq{cb}_{ic}",
                               bufs=bufs)
                for dc in range(DC):
                    nc.tensor.matmul(pq, wqk_sb[:, dc, cb * 128:(cb + 1) * 128],
                                     xts[dc][:, ic, :],
                                     start=(dc == 0), stop=(dc == DC - 1))
                dst = (ktp_sb[:, cb - 2, ic * 512:(ic + 1) * 512] if cb >= 2
                       else qt_sb[:, cb, ic * 512:(ic + 1) * 512])
                nc.vector.tensor_copy(dst, pq)

            def v_chunk(pool, it, tag="v", bufs=1):
                pv = pool.tile([128, 4, 64], f32, tag=tag, name=f"pv{it}",
                               bufs=bufs)
                for dc in range(DC):
                    nc.tensor.matmul(pv, xts[dc][:, it // 4, (it % 4) * 128:
                                                 (it % 4) * 128 + 128],
                                     wv_sb[:, dc, :],
                                     start=(dc == 0), stop=(dc == DC - 1))
                nc.vector.tensor_copy(vsb[:, it, :, :], pv)

            for ic in range(NI):
                qk_chunk(pfA, 2, ic)
            for ic in range(NI):
                qk_chunk(pfA, 0, ic)
            for it in range(NJ):
                v_chunk(pfA, it, bufs=2)

            # gate MLP (tiny, plain f32 matmuls); wfg1 streamed per d-chunk
            wfs = []
            for dc in range(DC):
                wf = wfp.tile([128, 256], f32, tag="wfg1", name=f"wf{dc}")
                nc.scalar.dma_start(out=wf, in_=WFG1[dc * 128:(dc + 1) * 128, :])
                wfs.append(wf)
            pg = pft.tile([128, 512], f32, tag="tiny", name="pg")
            for mc in range(2):
                for dc in range(DC):
                    nc.tensor.matmul(pg[:, mc:mc + 1],
                                     wfs[dc][:, mc * 128:(mc + 1) * 128],
                                     pooled_sb[:, dc:dc + 1],
                                     start=(dc == 0), stop=(dc == DC - 1))
            for mc in range(2):
                nc.scalar.activation(out=hidg_sb[:, mc:mc + 1], in_=pg[:, mc:mc + 1],
                                     func=AFT.Silu, bias=bfg1_sb[:, mc:mc + 1],
                                     scale=1.0 / n)
            pgp = pft.tile([128, 512], f32, tag="tiny")
            for mc in range(2):
                nc.tensor.matmul(pgp[0:1, 0:1], hidg_sb[:, mc:mc + 1],
                                 wfg2_sb[:, mc:mc + 1],
                                 start=(mc == 0), stop=(mc == 1))
            nc.scalar.activation(out=g_sb, in_=pgp[0:1, 0:1], func=AFT.Sigmoid,
                                 bias=bfg2_sb, scale=1.0)
            pgb = pft.tile([128, 512], f32, tag="tiny")
            nc.tensor.matmul(pgb[:, 0:1], ones_f, g_sb, start=True, stop=True)
            nc.vector.tensor_copy(gb_sb, pgb[:, 0:1])

            # coverage MLP (tiny, plain f32): hidden^T then cov (scaled by g)
            hidc_sb = big2.tile([128, 2, n], bf16, tag="big", name="hidc")
            for mc in range(2):
                for jc in range(NI):
                    ph = pft.tile([128, 512], f32, tag="tiny")
                    nc.tensor.matmul(ph, wce1_sb[:, mc * 128:(mc + 1) * 128],
                                     covT_sb[:, jc * 512:(jc + 1) * 512],
                                     start=True, stop=True)
                    nc.scalar.activation(out=hidc_sb[:, mc, jc * 512:(jc + 1) * 512],
                                         in_=ph, func=AFT.Silu,
                                         bias=bce1_sb[:, mc:mc + 1], scale=1.0)
            for jt in range(NJ):
                pc = pft.tile([128, 512], f32, tag="tiny")
                for mc in range(2):
                    nc.tensor.matmul(pc[:, 0:4], hidc_sb[:, mc, jt * 128:(jt + 1) * 128],
                                     wce2_sb[:, mc * 4:(mc + 1) * 4],
                                     start=(mc == 0), stop=(mc == 1))
                nc.vector.tensor_add(bias_sb[:, jt, :], pc[:, 0:4], bce2_sb)
            for jt in range(NJ):
                nc.vector.tensor_scalar_mul(out=bias_sb[:, jt, :],
                                            in0=bias_sb[:, jt, :], scalar1=gb_sb)

        # ---- attention stream with interleaved jobs ----
        scale = float(HD) ** -0.5
        attn_sb = big2.tile([128, 2, n], bf16, tag="big", name="attn")
        with tc.tile_pool(name="pop", bufs=1, space="PSUM") as pop, \
             tc.tile_pool(name="pdp", bufs=1, space="PSUM") as pdp, \
             tc.tile_pool(name="pjp", bufs=1, space="PSUM") as pjp, \
             tc.tile_pool(name="pss", bufs=2, space="PSUM") as pss:

            # Job schedule: one 2-matmul piece per j-tile window, so each
            # insertion costs ~430ns of PE queue time. K^T(p1) pieces fill
            # block 0, Q^T(p1) pieces block 1 (both consumed from block 2 on);
            # ic0 out-projection halves fill block 3.
            def qk_pieces(cb, ic):
                state = {}

                def mk(dc0):
                    def f():
                        if dc0 == 0:
                            state["pq"] = pjp.tile([128, 512], f32, tag="pj",
                                                   name=f"jq{cb}_{ic}", bufs=1)
                        pq = state["pq"]
                        for dc in (dc0, dc0 + 1):
                            nc.tensor.matmul(
                                pq, wqk_sb[:, dc, cb * 128:(cb + 1) * 128],
                                xts[dc][:, ic, :],
                                start=(dc == 0), stop=(dc == DC - 1))
                        if dc0 == DC - 2:
                            dst = (ktp_sb[:, cb - 2, ic * 512:(ic + 1) * 512]
                                   if cb >= 2 else
                                   qt_sb[:, cb, ic * 512:(ic + 1) * 512])
                            nc.vector.tensor_copy(dst, pq)
                    return f
                return [mk(d) for d in range(0, DC, 2)]

            jobs = []
            for ic in range(NI):
                jobs.extend(qk_pieces(3, ic))
            for ic in range(NI):
                jobs.extend(qk_pieces(1, ic))

            def py_job(it, half):
                py = pjp.tile([128, 512], f32, tag="pj", name=f"py{it}_{half}",
                              bufs=1)
                for pt in range(2):
                    nc.tensor.matmul(py,
                                     attn_sb[:, pt, it * 128:(it + 1) * 128],
                                     wo_sb[:, pt, half * 512:(half + 1) * 512],
                                     start=(pt == 0), stop=(pt == 1))
                yh = yp.tile([128, 512], bf16, tag="y_sb", name=f"yh{it}_{half}")
                nc.vector.tensor_copy(yh, py)
                nc.sync.dma_start(out=OUT[it * 128:(it + 1) * 128,
                                          half * 512:(half + 1) * 512], in_=yh)

            done_its = 0
            for bi, (p, ic) in enumerate([(0, 0), (0, 1), (1, 0), (1, 1)]):
                po = pop.tile([128, IC_W], f32, tag="o", name=f"po{p}_{ic}")
                pd = pdp.tile([128, 512], f32, tag="d", name=f"pd{p}_{ic}")
                for jt in range(NJ):
                    js = slice(jt * 128, (jt + 1) * 128)
                    pss_t, es = [], []
                    for hh in range(2):
                        lo = hh * 64
                        ps_ = pss.tile([128, IC_W], f32, tag="s",
                                       name=f"s{p}_{ic}_{jt}_{hh}")
                        for q in range(IC_W // 512):
                            nc.tensor.matmul(
                                ps_[:, q * 512:(q + 1) * 512],
                                ktp_sb[lo:lo + 64, p, js],
                                qt_sb[lo:lo + 64, p,
                                      ic * IC_W + q * 512:ic * IC_W + (q + 1) * 512],
                                start=True, stop=True)
                        pss_t.append(ps_)
                    for hh in range(2):
                        h = 2 * p + hh
                        e = ep.tile([128, IC_W], bf16, tag="e",
                                    name=f"e{p}_{ic}_{jt}_{hh}")
                        nc.scalar.activation(out=e, in_=pss_t[hh], func=AFT.Exp,
                                             bias=bias_sb[:, jt, h:h + 1],
                                             scale=scale)
                        es.append(e)
                    st, sp = (jt == 0), (jt == NJ - 1)
                    for hh in range(2):
                        lo = hh * 64
                        h = 2 * p + hh
                        for q in range(IC_W // 512):
                            nc.tensor.matmul(
                                po[lo:lo + 64, q * 512:(q + 1) * 512],
                                vsb[:, jt, h, :],
                                es[hh][:, q * 512:(q + 1) * 512],
                                start=st, stop=sp)
                        dbase = 64 if hh == 0 else 0
                        for q in range(IC_W // 512):
                            dp = dbase + 32 * q
                            nc.tensor.matmul(
                                pd[dp:dp + 1, :],
                                ones_bf,
                                es[hh][:, q * 512:(q + 1) * 512],
                                start=st, stop=sp,
                                tile_position=(0, dp))
                    if jobs:
                        jobs.pop(0)()
                # normalize: 1/d broadcast over the 64 v-rows of each head
                osl = slice(ic * IC_W, (ic + 1) * IC_W)
                for hh in range(2):
                    lo = hh * 64
                    dbase = 64 if hh == 0 else 0
                    dd = rp.tile([1, IC_W], f32, tag="dd",
                                 name=f"dd{p}_{ic}_{hh}")
                    nc.vector.tensor_copy(dd[:, 0:512], pd[dbase:dbase + 1, :])
                    nc.vector.tensor_copy(dd[:, 512:1024],
                                          pd[dbase + 32:dbase + 33, :])
                    rr = rp.tile([1, IC_W], f32, tag="rr",
                                 name=f"rr{p}_{ic}_{hh}")
                    nc.vector.reciprocal_approx_fast(out=rr, in_=dd)
                    recb = rp.tile([64, IC_W], f32, tag="recb",
                                   name=f"recb{p}_{ic}_{hh}")
                    nc.gpsimd.partition_broadcast(recb, rr)
                    nc.vector.tensor_mul(attn_sb[lo:lo + 64, p, osl],
                                         po[lo:lo + 64, :], recb)
                if bi == 2:
                    # attn for ic0 complete (both pairs): out-projection of
                    # its 0..7 becomes interleavable during block 3
                    for it in range(NJ // 2):
                        for half in range(2):
                            jobs.append(lambda it=it, half=half: py_job(it, half))
                    done_its = NJ // 2

        # ---- tail: remaining out-projection with a wide PSUM pool ----
        with tc.tile_pool(name="psy", bufs=2, space="PSUM") as psy:
            for it in range(done_its, NJ):
                py = psy.tile([128, D], f32, tag="y")
                for pt in range(2):
                    for half in range(2):
                        nc.tensor.matmul(
                            py[:, half * 512:(half + 1) * 512],
                            attn_sb[:, pt, it * 128:(it + 1) * 128],
                            wo_sb[:, pt, half * 512:(half + 1) * 512],
                            start=(pt == 0), stop=(pt == 1))
                y_sb = yp.tile([128, D], bf16, tag="y_sb", name=f"ysb{it}")
                if it % 2 == 0:
                    nc.vector.tensor_copy(y_sb, py)
                else:
                    nc.scalar.copy(y_sb, py)
                nc.sync.dma_start(out=OUT[it * 128:(it + 1) * 128, :], in_=y_sb)

    nc.compile()
    return nc


def make_in_maps(x, coverage, w_qkv, w_out, b_out, w_ce1, b_ce1, w_ce2, b_ce2,
                 w_fg1, b_fg1, w_fg2, b_fg2, n=N):
    f = np.float32
    x = np.asarray(x, f)
    coverage = np.asarray(coverage, f)
    w_qkv = np.asarray(w_qkv, f)
    w_out = np.asarray(w_out, f)
    in_maps = []
    for c in range(NCORES):
        b, hg = divmod(c, 4)
        cs, ce = hg * 256, (hg + 1) * 256
        wq = w_qkv[:, 0 * D + cs:0 * D + ce]
        wk = w_qkv[:, 1 * D + cs:1 * D + ce]
        wv = w_qkv[:, 2 * D + cs:2 * D + ce]
        m = {
            "xT": _bf16(x[b].T),
            "wqk": _bf16(np.concatenate([wq, wk], axis=1)),
            "wv": _bf16(wv),
            "wo": _bf16(w_out[cs:ce, :]),
            "covT": _bf16(coverage[b, :, 0][None, :]),
            "wce1": _bf16(w_ce1),
            "bce1": np.ascontiguousarray(np.asarray(b_ce1, f).reshape(2, 128).T),
            "wce2": _bf16(
                np.asarray(w_ce2, f)[:, 4 * hg:4 * hg + 4].reshape(2, 128, 4)
                .transpose(1, 0, 2).reshape(128, 8)),
            "bce2": np.tile(np.asarray(b_ce2, f)[4 * hg:4 * hg + 4][None, :], (128, 1)),
            "wfg1": np.ascontiguousarray(np.asarray(w_fg1, f)),
            "bfg1": np.ascontiguousarray(np.asarray(b_fg1, f).reshape(2, 128).T),
            "wfg2": np.ascontiguousarray(np.asarray(w_fg2, f).reshape(2, 128).T),
            "bfg2": np.asarray(b_fg2, f).reshape(1, 1),
        }
        in_maps.append(m)
    return in_maps


def kernel(**inputs):
    from concourse.bass_utils import run_bass_kernel_spmd
    if "nc" not in _COMPILED:
        _COMPILED["nc"] = build(N)
    nc = _COMPILED["nc"]
    in_maps = make_in_maps(**inputs)
    res = run_bass_kernel_spmd(nc, in_maps, core_ids=list(range(NCORES)))
    outs = [np.asarray(res.results[c]["out"], dtype=np.float32)
            for c in range(NCORES)]
    b_out = np.asarray(inputs["b_out"], np.float32)
    full = np.stack([
        outs[0] + outs[1] + outs[2] + outs[3] + b_out[None, :],
        outs[4] + outs[5] + outs[6] + outs[7] + b_out[None, :],
    ]).astype(np.float32)
    return full
